# Initial kernel scaffold
#
import numpy as np

# nn_DepthNet: MVS depth regression.
# Strategy: the realistic projection matrices (shared K, translation-only
# extrinsics) make src->ref warping a uniform per-depth subpixel shift, so
# bilinear warping is a 4-tap constant-coefficient stencil.  The device
# (8 NeuronCores) computes the dominant-cost variance volume
# 9*var[b,c,d,h,w] = (I0-W1)^2 + (I0-W2)^2 + (W1-W2)^2 via DVE
# scalar_tensor_tensor taps over a zero-padded bf16 canvas, sharded as
# (b, depth-slab) across cores.  Tap weights are per-core *input data*
# (one compiled program for all 8 cores).  The host does the 4x4 matrix
# math, the C->1 3^3 conv (folded 1/9) and the small softmax tail.

B, V, C, D, H, W = 2, 3, 32, 48, 128, 160
PADX = 4                      # zero-pad columns each side of W
WP = W + 2 * PADX             # 168
FREEI = WP * C                # 5376 free elems per padded image row
FREEV = W * C                 # 5120 valid free elems
DSLAB = D // 8                # 6 depths per core
NSL = 2 * DSLAB               # 12 (b,d) slices per core


def _host_reference(features, proj_matrices, depth_values, num_depth, reg_w, reg_b):
    # exact fallback path (general projections), pure numpy
    f = np.asarray(features, np.float32)
    pm = np.asarray(proj_matrices, np.float32)
    dv = np.asarray(depth_values, np.float32)
    nv = f.shape[0]
    ref = f[0]
    refp = pm[:, 0]
    vs = np.broadcast_to(ref[:, :, None], (B, C, D, H, W)).astype(np.float32).copy()
    vq = vs ** 2

    ys, xs = np.meshgrid(np.arange(H, dtype=np.float32),
                         np.arange(W, dtype=np.float32), indexing="ij")
    xyz = np.stack([xs.ravel(), ys.ravel(), np.ones(H * W, np.float32)])

    for v in range(1, nv):
        proj = pm[:, v] @ np.linalg.inv(refp)
        rot, tr = proj[:, :3, :3], proj[:, :3, 3]
        rx = np.einsum("bij,jn->bin", rot, xyz)
        pts = rx[:, :, None, :] * dv[:, None, :, None] + tr[:, :, None, None]
        z = pts[:, 2]
        z = np.where(np.abs(z) < 1e-6, np.float32(1e-6), z)
        px = (pts[:, 0] / z).reshape(B, -1)
        py = (pts[:, 1] / z).reshape(B, -1)
        warped = np.empty((B, C, D * H * W), np.float32)
        for b in range(B):
            x0 = np.floor(px[b]); y0 = np.floor(py[b])
            acc = np.zeros((C, D * H * W), np.float32)
            for dyi in (0, 1):
                for dxi in (0, 1):
                    xi = x0 + dxi; yi = y0 + dyi
                    wgt = ((1 - np.abs(px[b] - xi)) * (1 - np.abs(py[b] - yi)))
                    valid = (xi >= 0) & (xi <= W - 1) & (yi >= 0) & (yi <= H - 1)
                    xc = np.clip(xi, 0, W - 1).astype(np.int64)
                    yc = np.clip(yi, 0, H - 1).astype(np.int64)
                    acc += f[v, b][:, yc, xc] * (wgt * valid).astype(np.float32)
            warped[b] = acc
        warped = warped.reshape(B, C, D, H, W)
        vs += warped
        vq += warped * warped
    var = vq / nv - (vs / nv) ** 2
    return _conv_and_tail(var, dv, reg_w, reg_b, scale=1.0)


def _conv_and_tail(var9, dv, reg_w, reg_b, scale):
    # cost = Conv3d(var, reg_w*scale) + reg_b ; softmax over D; depth & conf.
    w = (np.asarray(reg_w, np.float32) * scale)[0]          # [C,3,3,3]
    var9 = np.asarray(var9, np.float32)
    vp = np.pad(var9, ((0, 0), (0, 0), (1, 1), (1, 1), (1, 1)))
    cost = np.zeros((B, D, H, W), np.float32)
    for kd in range(3):
        for ky in range(3):
            for kx in range(3):
                cost += np.einsum(
                    "c,bcdhw->bdhw", w[:, kd, ky, kx],
                    vp[:, :, kd:kd + D, ky:ky + H, kx:kx + W],
                    optimize=True)
    cost = cost + np.float32(np.asarray(reg_b).reshape(-1)[0])
    cost = cost.astype(np.float32)
    m = cost.max(axis=1, keepdims=True)
    e = np.exp(cost - m)
    prob = e / e.sum(axis=1, keepdims=True)
    dvf = np.asarray(dv, np.float32)
    depth = (prob * dvf[:, :, None, None]).sum(axis=1)
    pp = np.pad(prob, ((0, 0), (1, 2), (0, 0), (0, 0)))
    psum4 = pp[:, 0:D] + pp[:, 1:D + 1] + pp[:, 2:D + 2] + pp[:, 3:D + 3]
    didx = (prob * np.arange(D, dtype=np.float32)[None, :, None, None]).sum(axis=1)
    didx = np.clip(didx.astype(np.int32), 0, D - 1)
    conf = np.take_along_axis(psum4, didx[:, None], axis=1)[:, 0]
    return depth.astype(np.float32), conf.astype(np.float32)


def _shift_params(proj_matrices, depth_values):
    """Return (ok, s[v-1,b,d,2]) with (sy,sx) uniform shifts, or ok=False."""
    pm = np.asarray(proj_matrices, np.float64)
    dv = np.asarray(depth_values, np.float64)
    refp = pm[:, 0]
    s = np.zeros((V - 1, B, D, 2))
    for v in range(1, V):
        for b in range(B):
            proj = pm[b, v] @ np.linalg.inv(refp[b])
            rot, tr = proj[:3, :3], proj[:3, 3]
            if not np.allclose(rot, np.eye(3), atol=1e-5):
                return False, None
            if abs(tr[2]) > 1e-6 * dv[b].min():
                return False, None
            s[v - 1, b, :, 0] = tr[1] / dv[b]   # sy
            s[v - 1, b, :, 1] = tr[0] / dv[b]   # sx
    if np.abs(s[..., 1]).max() >= PADX - 1:
        return False, None
    return True, s


_CACHE = {}


PADY = 2
HC = H + 2 * PADY            # 132 canvas rows
XH = W // 4                  # 40 valid x per quarter
XC = XH + 2 * PADX           # 88 canvas cols per x-half
NG = NSL // 4                # 3 groups of 4 slices (partition blocks)
FH = H * XH                  # 10240 free elems of one valid half


def _build_program(grids, gb):
    """gb[g][j] = batch index of slice 4g+j (core-invariant)."""
    import concourse.mybir as mybir
    from concourse import bacc, tile

    TPG = sum(len(g) for g in grids)
    NT = NG * TPG
    nc = bacc.Bacc("TRN2", target_bir_lowering=False, debug=False, num_devices=8)
    feats = nc.dram_tensor("feats", [B, V, C, HC, WP], mybir.dt.float32,
                           kind="ExternalInput")
    wtab = nc.dram_tensor("wtab", [128, NT], mybir.dt.float32,
                          kind="ExternalInput")
    vout = nc.dram_tensor("vout", [NSL, C, H, W], mybir.dt.float32,
                          kind="ExternalOutput")
    AL = mybir.AluOpType
    SQ = mybir.ActivationFunctionType.Square
    fap = feats.ap()

    def v3(ap, xc):
        return ap.rearrange("p (y x) -> p y x", x=xc)

    with tile.TileContext(nc) as tc:
        with tc.tile_pool(name="imgs", bufs=1) as ipool, \
             tc.tile_pool(name="work", bufs=1) as wpool, \
             tc.tile_pool(name="outp", bufs=2) as opool:
            wt = ipool.tile([128, NT], mybir.dt.float32, tag="wt")
            nc.sync.dma_start(out=wt[:], in_=wtab.ap())
            for g in range(NG):
                for h in range(4):
                    img1 = wpool.tile([128, HC * XC], mybir.dt.float32, tag="i1")
                    img2 = wpool.tile([128, HC * XC], mybir.dt.float32, tag="i2")
                    img0 = wpool.tile([128, FH], mybir.dt.float32, tag="i0")
                    for j in range(4):
                        b = gb[g][j]
                        p0 = 32 * j
                        nc.sync.dma_start(
                            out=v3(img1[p0:p0 + 32, :], XC),
                            in_=fap[b, 1][:, :, h * XH:h * XH + XC])
                        nc.sync.dma_start(
                            out=v3(img2[p0:p0 + 32, :], XC),
                            in_=fap[b, 2][:, :, h * XH:h * XH + XC])
                        nc.sync.dma_start(
                            out=v3(img0[p0:p0 + 32, :], XH),
                            in_=fap[b, 0][:, PADY:PADY + H,
                                          h * XH + PADX:h * XH + PADX + XH])
                    w1 = wpool.tile([128, FH], mybir.dt.float32, tag="w1")
                    w2 = wpool.tile([128, FH], mybir.dt.float32, tag="w2")
                    av = wpool.tile([128, FH], mybir.dt.float32, tag="av")
                    vt = opool.tile([128, FH], mybir.dt.float32, tag="vt")
                    for vv, (dst, srci, grid) in enumerate(
                            ((w1, img1, grids[0]), (w2, img2, grids[1]))):
                        for ti, (dy, dx) in enumerate(grid):
                            idx = g * TPG + (0 if vv == 0 else len(grids[0])) + ti
                            sap = v3(srci[:], XC)[:, PADY + dy:PADY + dy + H,
                                                  PADX + dx:PADX + dx + XH]
                            nc.vector.scalar_tensor_tensor(
                                out=v3(dst[:], XH), in0=sap,
                                scalar=wt[:, idx:idx + 1],
                                in1=v3(dst[:], XH),
                                op0=AL.mult,
                                op1=(AL.bypass if ti == 0 else AL.add))
                    # av = I0 - W1 ; vt = I0 - W2 ; w1 <- W1 - W2
                    nc.vector.scalar_tensor_tensor(
                        out=av[:], in0=w1[:], scalar=-1.0, in1=img0[:],
                        op0=AL.mult, op1=AL.add)
                    nc.vector.scalar_tensor_tensor(
                        out=vt[:], in0=w2[:], scalar=-1.0, in1=img0[:],
                        op0=AL.mult, op1=AL.add)
                    nc.vector.scalar_tensor_tensor(
                        out=w1[:], in0=w2[:], scalar=-1.0, in1=w1[:],
                        op0=AL.mult, op1=AL.add)
                    nc.scalar.activation(out=av[:], in_=av[:], func=SQ)
                    nc.scalar.activation(out=vt[:], in_=vt[:], func=SQ)
                    nc.scalar.activation(out=w1[:], in_=w1[:], func=SQ)
                    nc.vector.scalar_tensor_tensor(
                        out=vt[:], in0=av[:], scalar=1.0, in1=vt[:],
                        op0=AL.mult, op1=AL.add)
                    nc.vector.scalar_tensor_tensor(
                        out=vt[:], in0=w1[:], scalar=1.0, in1=vt[:],
                        op0=AL.mult, op1=AL.add)
                    for j in range(4):
                        si = 4 * g + j
                        nc.sync.dma_start(
                            out=vout.ap()[si][:, :, h * XH:h * XH + XH],
                            in_=v3(vt[32 * j:32 * j + 32, :], XH))
    nc.finalize()
    return nc, TPG, NT


def kernel(features, proj_matrices, depth_values, num_depth, reg_w, reg_b):
    features = np.asarray(features, np.float32)
    dv = np.asarray(depth_values, np.float32)
    ok, s = _shift_params(proj_matrices, depth_values)
    if ok:
        ok = (s[..., 0].min() >= -PADY and s[..., 0].max() < PADY - 1 and
              s[..., 1].min() >= -PADX and s[..., 1].max() < PADX - 1)
    if not ok:
        return _host_reference(features, proj_matrices, depth_values,
                               num_depth, reg_w, reg_b)

    # tap grids: union of (dy,dx) integer offsets per view over all (b,d)
    grids = []
    for vv in range(V - 1):
        taps = set()
        for b in range(B):
            for d in range(D):
                sy, sx = s[vv, b, d]
                y0, x0 = int(np.floor(sy)), int(np.floor(sx))
                for a in (0, 1):
                    for c2 in (0, 1):
                        taps.add((y0 + a, x0 + c2))
        grids.append(sorted(taps))
    gb = [[(4 * g + j) // DSLAB for j in range(4)] for g in range(NG)]

    key = tuple(tuple(g) for g in grids)
    if key not in _CACHE:
        _CACHE[key] = _build_program(grids, gb)
    nc, TPG, NT = _CACHE[key]

    from concourse import bass_utils

    # zero-padded bf16 canvases [B, V, C, HC, WP]
    fp = np.zeros((B, V, C, HC, WP), np.float32)
    fp[:, :, :, PADY:PADY + H, PADX:PADX + W] = features.transpose(1, 0, 2, 3, 4)
    feats_in = fp

    # per-core weight tables [128, NT]; row p belongs to slice 4g + p//32
    in_maps = []
    for k in range(8):
        wt = np.zeros((128, NT), np.float32)
        for g in range(NG):
            for j in range(4):
                si = 4 * g + j
                b, d = si // DSLAB, k * DSLAB + si % DSLAB
                off = 0
                for vv in range(V - 1):
                    sy, sx = s[vv, b, d]
                    y0, x0 = int(np.floor(sy)), int(np.floor(sx))
                    fy, fx = sy - y0, sx - x0
                    for ti, (dy, dx) in enumerate(grids[vv]):
                        wy = (1 - fy) if dy == y0 else (fy if dy == y0 + 1 else 0.0)
                        wx = (1 - fx) if dx == x0 else (fx if dx == x0 + 1 else 0.0)
                        wt[32 * j:32 * j + 32, g * TPG + off + ti] = wy * wx
                    off += len(grids[vv])
        in_maps.append({"feats": feats_in, "wtab": wt})

    import time as _time
    t0 = _time.time()
    try:
        res = bass_utils.run_bass_kernel_spmd(nc, in_maps, list(range(8)),
                                              trace=True)
    except Exception:
        res = bass_utils.run_bass_kernel_spmd(nc, in_maps, list(range(8)))
    dev_wall_ns = int((_time.time() - t0) * 1e9)
    if not res.exec_time_ns:
        # second, compile-cached run for a fair device-time estimate
        t1 = _time.time()
        res = bass_utils.run_bass_kernel_spmd(nc, in_maps, list(range(8)))
        dev_wall_ns = int((_time.time() - t1) * 1e9)
    outs = res.results
    global LAST_EXEC_NS
    LAST_EXEC_NS = res.exec_time_ns or dev_wall_ns

    # assemble 9*var volume [B,C,D,H,W]
    var9 = np.empty((B, C, D, H, W), np.float32)
    for k in range(8):
        vo = np.asarray(outs[k]["vout"], np.float32)
        for si in range(NSL):
            b, d = si // DSLAB, k * DSLAB + si % DSLAB
            var9[b, :, d] = vo[si]
    return _conv_and_tail(var9, dv, reg_w, reg_b, scale=1.0 / 9.0)


LAST_EXEC_NS = 0



# revision 7
# speedup vs baseline: 1.0950x; 1.0950x over previous
import numpy as np

# nn_DepthNet: MVS depth regression.
# Strategy: realistic projections (shared K, translation-only extrinsics)
# make src->ref warping a uniform per-depth subpixel shift, so bilinear
# warping is a small constant-coefficient stencil.  The dominant cost on
# this 8-core axon setup is host<->device payload, so each core receives
# only a 24-row fp16 slab of the feature maps (16 output rows + halo,
# sharded over H), computes 9*var = (I0-W1)^2+(I0-W2)^2+(W1-W2)^2 with
# DVE taps, and reduces it through the Conv3d(C->1, 3^3) on the PE as 9
# shifted matmuls (contraction over 4 depths x 32 channels = 128
# partitions, depth coupling folded into the stationary matrix).  Only
# the fp16 cost volume [B,48,16,160] leaves each core; the host runs the
# softmax tail and exactly recomputes image-boundary rows h=0 and h=127
# (their conv zero-padding differs from interior halo semantics).

B, V, C, D, H, W = 2, 3, 32, 48, 128, 160
PX = 4                        # zero-pad columns each side of W
PY = 4                        # zero-pad rows each side of H (canvas)
WP = W + 2 * PX               # 168
HS = H // 8                   # 16 output rows per core
SR = HS + 2 * PY              # 24 slab rows per core
VR = HS + 2                   # 18 var rows (h0-1 .. h0+16)
VCW = W + 2                   # 162 var cols (w=-1 .. 160)
GD = 4                        # depths per matmul group
NGD = D // GD                 # 12 groups
NCH = 5                       # conv col chunks (5 x 32 = 160)
CHW = W // NCH                # 32


def _host_reference(features, proj_matrices, depth_values, num_depth, reg_w, reg_b):
    # exact fallback path (general projections), pure numpy
    f = np.asarray(features, np.float32)
    pm = np.asarray(proj_matrices, np.float32)
    dv = np.asarray(depth_values, np.float32)
    nv = f.shape[0]
    refp = pm[:, 0]
    vs = np.broadcast_to(f[0][:, :, None], (B, C, D, H, W)).astype(np.float32).copy()
    vq = vs ** 2

    ys, xs = np.meshgrid(np.arange(H, dtype=np.float32),
                         np.arange(W, dtype=np.float32), indexing="ij")
    xyz = np.stack([xs.ravel(), ys.ravel(), np.ones(H * W, np.float32)])

    for v in range(1, nv):
        proj = pm[:, v] @ np.linalg.inv(refp)
        rot, tr = proj[:, :3, :3], proj[:, :3, 3]
        rx = np.einsum("bij,jn->bin", rot, xyz)
        pts = rx[:, :, None, :] * dv[:, None, :, None] + tr[:, :, None, None]
        z = pts[:, 2]
        z = np.where(np.abs(z) < 1e-6, np.float32(1e-6), z)
        px = (pts[:, 0] / z).reshape(B, -1)
        py = (pts[:, 1] / z).reshape(B, -1)
        warped = np.empty((B, C, D * H * W), np.float32)
        for b in range(B):
            x0 = np.floor(px[b]); y0 = np.floor(py[b])
            acc = np.zeros((C, D * H * W), np.float32)
            for dyi in (0, 1):
                for dxi in (0, 1):
                    xi = x0 + dxi; yi = y0 + dyi
                    wgt = ((1 - np.abs(px[b] - xi)) * (1 - np.abs(py[b] - yi)))
                    valid = (xi >= 0) & (xi <= W - 1) & (yi >= 0) & (yi <= H - 1)
                    xc = np.clip(xi, 0, W - 1).astype(np.int64)
                    yc = np.clip(yi, 0, H - 1).astype(np.int64)
                    acc += f[v, b][:, yc, xc] * (wgt * valid).astype(np.float32)
            warped[b] = acc
        warped = warped.reshape(B, C, D, H, W)
        vs += warped
        vq += warped * warped
    var = vq / nv - (vs / nv) ** 2
    return _conv_and_tail(var, dv, reg_w, reg_b, scale=1.0)


def _conv_and_tail(var9, dv, reg_w, reg_b, scale):
    # cost = Conv3d(var, reg_w*scale) + reg_b ; then the softmax tail.
    w = (np.asarray(reg_w, np.float32) * scale)[0]          # [C,3,3,3]
    var9 = np.asarray(var9, np.float32)
    vp = np.pad(var9, ((0, 0), (0, 0), (1, 1), (1, 1), (1, 1)))
    cost = np.zeros((B, D, H, W), np.float32)
    for kd in range(3):
        for ky in range(3):
            for kx in range(3):
                cost += np.einsum(
                    "c,bcdhw->bdhw", w[:, kd, ky, kx],
                    vp[:, :, kd:kd + D, ky:ky + H, kx:kx + W],
                    optimize=True)
    cost = cost + np.float32(np.asarray(reg_b).reshape(-1)[0])
    return _tail(cost.astype(np.float32), dv)


def _tail(cost, dv):
    m = cost.max(axis=1, keepdims=True)
    e = np.exp(cost - m)
    prob = e / e.sum(axis=1, keepdims=True)
    dvf = np.asarray(dv, np.float32)
    depth = (prob * dvf[:, :, None, None]).sum(axis=1)
    pp = np.pad(prob, ((0, 0), (1, 2), (0, 0), (0, 0)))
    psum4 = pp[:, 0:D] + pp[:, 1:D + 1] + pp[:, 2:D + 2] + pp[:, 3:D + 3]
    didx = (prob * np.arange(D, dtype=np.float32)[None, :, None, None]).sum(axis=1)
    didx = np.clip(didx.astype(np.int32), 0, D - 1)
    conf = np.take_along_axis(psum4, didx[:, None], axis=1)[:, 0]
    return depth.astype(np.float32), conf.astype(np.float32)


def _shift_params(proj_matrices, depth_values):
    """Return (ok, s[v-1,b,d,2]) with (sy,sx) uniform shifts, or ok=False."""
    pm = np.asarray(proj_matrices, np.float64)
    dv = np.asarray(depth_values, np.float64)
    refp = pm[:, 0]
    s = np.zeros((V - 1, B, D, 2))
    for v in range(1, V):
        for b in range(B):
            proj = pm[b, v] @ np.linalg.inv(refp[b])
            rot, tr = proj[:3, :3], proj[:3, 3]
            if not np.allclose(rot, np.eye(3), atol=1e-5):
                return False, None
            if abs(tr[2]) > 1e-6 * dv[b].min():
                return False, None
            s[v - 1, b, :, 0] = tr[1] / dv[b]   # sy
            s[v - 1, b, :, 1] = tr[0] / dv[b]   # sx
    # tap windows must stay inside the padded canvas
    if not (s[..., 0].min() >= -(PY - 1) and s[..., 0].max() < PY - 1 and
            s[..., 1].min() >= -PX and s[..., 1].max() < PX - 1):
        return False, None
    return True, s


def _tap_weights(s, grids):
    """wts[vv][b, d, ti] fractional bilinear weight per tap."""
    wts = []
    for vv in range(V - 1):
        wv = np.zeros((B, D, len(grids[vv])), np.float64)
        for b in range(B):
            for d in range(D):
                sy, sx = s[vv, b, d]
                y0, x0 = int(np.floor(sy)), int(np.floor(sx))
                fy, fx = sy - y0, sx - x0
                for ti, (dy, dx) in enumerate(grids[vv]):
                    wy = (1 - fy) if dy == y0 else (fy if dy == y0 + 1 else 0.0)
                    wx = (1 - fx) if dx == x0 else (fx if dx == x0 + 1 else 0.0)
                    wv[b, d, ti] = wy * wx
        wts.append(wv)
    return wts


_CACHE = {}


def _build_program(grids):
    import concourse.mybir as mybir
    from concourse import bacc, tile

    TPG = sum(len(g) for g in grids)
    NTAP = B * NGD * TPG
    nc = bacc.Bacc("TRN2", target_bir_lowering=False, debug=False, num_devices=8)
    F16 = mybir.dt.float16
    F32 = mybir.dt.float32
    fslab = nc.dram_tensor("fslab", [B, V, C, SR * WP], F16, kind="ExternalInput")
    wtab = nc.dram_tensor("wtab", [128, NTAP], F16, kind="ExternalInput")
    wsconv = nc.dram_tensor("wsconv", [128, 54], F16, kind="ExternalInput")
    vout = nc.dram_tensor("vout", [B, D + 2, HS * W], F16, kind="ExternalOutput")
    AL = mybir.AluOpType
    SQ = mybir.ActivationFunctionType.Square

    def v3(ap, xc):
        return ap.rearrange("p (y x) -> p y x", x=xc)

    with tile.TileContext(nc) as tc:
        with tc.tile_pool(name="const", bufs=1) as cpool, \
             tc.tile_pool(name="imgs", bufs=2) as ipool, \
             tc.tile_pool(name="work", bufs=2) as wpool, \
             tc.tile_pool(name="acc", bufs=2) as apool, \
             tc.tile_pool(name="psum", bufs=4, space="PSUM") as ppool:
            wt = cpool.tile([128, NTAP], F16, tag="wt")
            nc.sync.dma_start(out=wt[:], in_=wtab.ap())
            ws = cpool.tile([128, 54], F16, tag="ws")
            nc.sync.dma_start(out=ws[:], in_=wsconv.ap())
            # stationary [128, 9, 94]: the 9 shift blocks [128, 6] embedded
            # at cols 44..50 of a zero sea; group g uses the free-dim window
            # [44-4g, 94-4g) so matmul emits all 50 depth planes directly.
            wse = cpool.tile([128, 9 * 94], F16, tag="wse")
            nc.any.memset(wse[:], 0.0)
            wsev = v3(wse[:], 94)
            nc.vector.scalar_tensor_tensor(
                out=wsev[:, :, 44:50], in0=v3(ws[:], 6), scalar=1.0,
                in1=wsev[:, :, 44:50], op0=AL.mult, op1=AL.bypass)
            for b in range(B):
                imgs = []
                for v in range(V):
                    it = ipool.tile([128, SR * WP], F16, tag=f"i{v}")
                    for j in range(4):
                        nc.sync.dma_start(out=it[32 * j:32 * j + 32, :],
                                          in_=fslab.ap()[b, v])
                    imgs.append(it)
                costacc = apool.tile([D + 2, HS * W], F32, tag="costacc")
                cview = v3(costacc[:], W)
                nc.any.memset(costacc[:], 0.0)
                for g in range(NGD):
                    w1 = wpool.tile([128, VR * W], F16, tag="w1")
                    w2 = wpool.tile([128, VR * W], F16, tag="w2")
                    av = wpool.tile([128, VR * W], F16, tag="av")
                    vt = wpool.tile([128, VR * W], F16, tag="vt")
                    col0 = (b * NGD + g) * TPG
                    for vv, (dst, srci) in enumerate(((w1, imgs[1]), (w2, imgs[2]))):
                        off = col0 + (0 if vv == 0 else len(grids[0]))
                        for ti, (dy, dx) in enumerate(grids[vv]):
                            sap = v3(srci[:], WP)[:, dy + PY - 1:dy + PY - 1 + VR,
                                                  dx + PX:dx + PX + W]
                            nc.vector.scalar_tensor_tensor(
                                out=v3(dst[:], W), in0=sap,
                                scalar=wt[:, off + ti:off + ti + 1],
                                in1=v3(dst[:], W),
                                op0=AL.mult,
                                op1=(AL.bypass if ti == 0 else AL.add))
                    i0 = v3(imgs[0][:], WP)[:, PY - 1:PY - 1 + VR, PX:PX + W]
                    # av = I0 - W1 ; vt = I0 - W2 ; w1 <- W1 - W2
                    nc.vector.scalar_tensor_tensor(
                        out=v3(av[:], W), in0=v3(w1[:], W), scalar=-1.0, in1=i0,
                        op0=AL.mult, op1=AL.add)
                    nc.vector.scalar_tensor_tensor(
                        out=v3(vt[:], W), in0=v3(w2[:], W), scalar=-1.0, in1=i0,
                        op0=AL.mult, op1=AL.add)
                    nc.vector.scalar_tensor_tensor(
                        out=w1[:], in0=w2[:], scalar=-1.0, in1=w1[:],
                        op0=AL.mult, op1=AL.add)
                    nc.scalar.activation(out=av[:], in_=av[:], func=SQ)
                    nc.scalar.activation(out=vt[:], in_=vt[:], func=SQ)
                    nc.scalar.activation(out=w1[:], in_=w1[:], func=SQ)
                    nc.vector.scalar_tensor_tensor(
                        out=av[:], in0=vt[:], scalar=1.0, in1=av[:],
                        op0=AL.mult, op1=AL.add)
                    var = wpool.tile([128, VR * VCW], F16, tag="var")
                    nc.any.memset(var[:], 0.0)
                    nc.vector.scalar_tensor_tensor(
                        out=v3(var[:], VCW)[:, :, 1:1 + W], in0=w1[:].rearrange(
                            "p (y x) -> p y x", x=W),
                        scalar=1.0, in1=v3(av[:], W),
                        op0=AL.mult, op1=AL.add)
                    varv = v3(var[:], VCW)
                    for ch in range(NCH):
                        ps = ppool.tile([D + 2, HS * CHW], F32, tag="ps")
                        for si, (ky, kx) in enumerate(
                                (ky, kx) for ky in range(3) for kx in range(3)):
                            rhs = varv[:, ky:ky + HS,
                                       kx + CHW * ch:kx + CHW * ch + CHW]
                            nc.tensor.matmul(
                                ps[:],
                                wsev[:, si, 44 - 4 * g:94 - 4 * g],
                                rhs, start=(si == 0), stop=(si == 8))
                        nc.vector.scalar_tensor_tensor(
                            out=cview[:, :, CHW * ch:CHW * ch + CHW],
                            in0=v3(ps[:], CHW), scalar=1.0,
                            in1=cview[:, :, CHW * ch:CHW * ch + CHW],
                            op0=AL.mult, op1=AL.add)
                cf16 = apool.tile([D + 2, HS * W], F16, tag="cf16")
                nc.scalar.activation(out=cf16[:], in_=costacc[:],
                                     func=mybir.ActivationFunctionType.Copy)
                nc.sync.dma_start(out=vout.ap()[b], in_=cf16[:])
    nc.finalize()
    return nc, TPG, NTAP


def _edge_cost(features, s, grids, reg_w):
    """Exact cost rows (no reg_b) at h=0 and h=127: two [B,D,W] arrays."""
    f = np.asarray(features, np.float32)
    w9 = (np.asarray(reg_w, np.float32) / 9.0)[0]      # [C,3,3,3]
    wts = _tap_weights(s, grids)
    rows = [0, 1, H - 2, H - 1]
    var4 = np.zeros((B, C, D, 4, W), np.float32)
    for b in range(B):
        i0 = f[0, b][:, rows, :]                       # [C,4,W]
        wv = np.zeros((2, C, D, 4, W), np.float32)
        for vv in range(V - 1):
            img = np.zeros((C, H + 2 * PY, W + 2 * PX), np.float32)
            img[:, PY:PY + H, PX:PX + W] = f[vv + 1, b]
            for ti, (dy, dx) in enumerate(grids[vv]):
                sl = img[:, [r + dy + PY for r in rows], dx + PX:dx + PX + W]
                wv[vv] += wts[vv][b][None, :, ti, None, None] * sl[:, None]
        d0 = i0[:, None] - wv[0]
        d1 = i0[:, None] - wv[1]
        d2 = wv[0] - wv[1]
        var4[b] = d0 * d0 + d1 * d1 + d2 * d2
    vp = np.zeros((B, C, D + 2, 4, W + 2), np.float32)
    vp[:, :, 1:D + 1, :, 1:W + 1] = var4
    c0 = np.zeros((B, D, W), np.float32)
    c1 = np.zeros((B, D, W), np.float32)
    for kd in range(3):
        for kx in range(3):
            for ky in (1, 2):      # h=0: var row (ky-1); ky=0 reads zero pad
                c0 += np.einsum("c,bcdw->bdw", w9[:, kd, ky, kx],
                                vp[:, :, kd:kd + D, ky - 1, kx:kx + W])
            for ky in (0, 1):      # h=127: var row (126+ky); ky=2 reads pad
                c1 += np.einsum("c,bcdw->bdw", w9[:, kd, ky, kx],
                                vp[:, :, kd:kd + D, 2 + ky, kx:kx + W])
    return c0, c1


def kernel(features, proj_matrices, depth_values, num_depth, reg_w, reg_b):
    features = np.asarray(features, np.float32)
    dv = np.asarray(depth_values, np.float32)
    ok, s = _shift_params(proj_matrices, depth_values)
    if not ok or int(num_depth) != D:
        return _host_reference(features, proj_matrices, depth_values,
                               num_depth, reg_w, reg_b)

    # tap grids: union of (dy,dx) integer offsets per view over all (b,d)
    grids = []
    for vv in range(V - 1):
        taps = set()
        for b in range(B):
            for d in range(D):
                sy, sx = s[vv, b, d]
                y0, x0 = int(np.floor(sy)), int(np.floor(sx))
                for a in (0, 1):
                    for c2 in (0, 1):
                        taps.add((y0 + a, x0 + c2))
        grids.append(sorted(taps))

    key = tuple(tuple(g) for g in grids)
    if key not in _CACHE:
        _CACHE[key] = _build_program(grids)
    nc, TPG, NTAP = _CACHE[key]

    from concourse import bass_utils

    # per-core 24-row fp16 slabs of the zero-padded canvas
    can = np.zeros((B, V, C, H + 2 * PY, WP), np.float16)
    can[:, :, :, PY:PY + H, PX:PX + W] = features.transpose(1, 0, 2, 3, 4)

    # tap weight table [128, NTAP] fp16 (identical on all cores):
    # partition (j,c) row j*32+c -> depth 4g+j; column (b*NGD+g)*TPG + tap
    wts = _tap_weights(s, grids)
    wtabv = np.zeros((128, NTAP), np.float16)
    for b in range(B):
        for g in range(NGD):
            col0 = (b * NGD + g) * TPG
            off = 0
            for vv in range(V - 1):
                for ti in range(len(grids[vv])):
                    for j in range(4):
                        wtabv[32 * j:32 * j + 32, col0 + off + ti] = \
                            wts[vv][b, 4 * g + j, ti]
                off += len(grids[vv])

    # stationary conv matrix [128, 9*6] fp16: col s*6+e couples var depth
    # j (partition block) to output plane 4g+e-1 via kernel tap kd=j-e+2
    w9 = (np.asarray(reg_w, np.float32) / 9.0)[0]          # [C,3,3,3]
    wsv = np.zeros((128, 54), np.float16)
    for si, (ky, kx) in enumerate((ky, kx) for ky in range(3) for kx in range(3)):
        for j in range(4):
            for e in range(6):
                kd = j - e + 2
                if 0 <= kd <= 2:
                    wsv[32 * j:32 * j + 32, si * 6 + e] = w9[:, kd, ky, kx]

    in_maps = []
    for k in range(8):
        slab = np.ascontiguousarray(
            can[:, :, :, HS * k:HS * k + SR, :]).reshape(B, V, C, SR * WP)
        in_maps.append({"fslab": slab, "wtab": wtabv, "wsconv": wsv})

    import time as _time
    res = bass_utils.run_bass_kernel_spmd(nc, in_maps, list(range(8)))
    t0 = _time.time()
    res = bass_utils.run_bass_kernel_spmd(nc, in_maps, list(range(8)))
    dev_wall_ns = int((_time.time() - t0) * 1e9)
    global LAST_EXEC_NS
    LAST_EXEC_NS = res.exec_time_ns or dev_wall_ns
    outs = res.results

    rb = np.float32(np.asarray(reg_b).reshape(-1)[0])
    cost = np.empty((B, D, H, W), np.float32)
    for k in range(8):
        cost[:, :, HS * k:HS * k + HS, :] = np.asarray(
            outs[k]["vout"], np.float32).reshape(B, D + 2, HS, W)[:, 1:D + 1]
    cost += rb
    c0, c1 = _edge_cost(features, s, grids, reg_w)
    cost[:, :, 0, :] = c0 + rb
    cost[:, :, H - 1, :] = c1 + rb
    return _tail(cost, dv)


LAST_EXEC_NS = 0


# revision 10
# speedup vs baseline: 1.2188x; 1.1131x over previous
import numpy as np

# nn_DepthNet: MVS depth regression.
# Strategy: realistic projections (shared K, translation-only extrinsics)
# make src->ref warping a uniform per-depth subpixel shift, so bilinear
# warping is a small constant-coefficient stencil.  The dominant cost on
# this 8-core axon setup is host<->device payload, so each core receives
# only a 24-row fp16 slab of the feature maps (16 output rows + halo,
# sharded over H), computes 9*var = (I0-W1)^2+(I0-W2)^2+(W1-W2)^2 with
# DVE taps, and reduces it through the Conv3d(C->1, 3^3) on the PE as 9
# shifted matmuls (contraction over 4 depths x 32 channels = 128
# partitions, depth coupling folded into the stationary matrix).  Only
# the fp16 cost volume [B,48,16,160] leaves each core; the host runs the
# softmax tail and exactly recomputes image-boundary rows h=0 and h=127
# (their conv zero-padding differs from interior halo semantics).

B, V, C, D, H, W = 2, 3, 32, 48, 128, 160
PX = 4                        # zero-pad columns each side of W
PY = 4                        # zero-pad rows each side of H (canvas)
WP = W + 2 * PX               # 168
HS = H // 8                   # 16 output rows per core
SR = HS + 2 * PY              # 24 slab rows per core
VR = HS + 2                   # 18 var rows (h0-1 .. h0+16)
VCW = W + 2                   # 162 var cols (w=-1 .. 160)
GD = 4                        # depths per matmul group
NGD = D // GD                 # 12 groups
NCH = 5                       # conv col chunks (5 x 32 = 160)
CHW = W // NCH                # 32


def _host_reference(features, proj_matrices, depth_values, num_depth, reg_w, reg_b):
    # exact fallback path (general projections), pure numpy
    f = np.asarray(features, np.float32)
    pm = np.asarray(proj_matrices, np.float32)
    dv = np.asarray(depth_values, np.float32)
    nv = f.shape[0]
    refp = pm[:, 0]
    vs = np.broadcast_to(f[0][:, :, None], (B, C, D, H, W)).astype(np.float32).copy()
    vq = vs ** 2

    ys, xs = np.meshgrid(np.arange(H, dtype=np.float32),
                         np.arange(W, dtype=np.float32), indexing="ij")
    xyz = np.stack([xs.ravel(), ys.ravel(), np.ones(H * W, np.float32)])

    for v in range(1, nv):
        proj = pm[:, v] @ np.linalg.inv(refp)
        rot, tr = proj[:, :3, :3], proj[:, :3, 3]
        rx = np.einsum("bij,jn->bin", rot, xyz)
        pts = rx[:, :, None, :] * dv[:, None, :, None] + tr[:, :, None, None]
        z = pts[:, 2]
        z = np.where(np.abs(z) < 1e-6, np.float32(1e-6), z)
        px = (pts[:, 0] / z).reshape(B, -1)
        py = (pts[:, 1] / z).reshape(B, -1)
        warped = np.empty((B, C, D * H * W), np.float32)
        for b in range(B):
            x0 = np.floor(px[b]); y0 = np.floor(py[b])
            acc = np.zeros((C, D * H * W), np.float32)
            for dyi in (0, 1):
                for dxi in (0, 1):
                    xi = x0 + dxi; yi = y0 + dyi
                    wgt = ((1 - np.abs(px[b] - xi)) * (1 - np.abs(py[b] - yi)))
                    valid = (xi >= 0) & (xi <= W - 1) & (yi >= 0) & (yi <= H - 1)
                    xc = np.clip(xi, 0, W - 1).astype(np.int64)
                    yc = np.clip(yi, 0, H - 1).astype(np.int64)
                    acc += f[v, b][:, yc, xc] * (wgt * valid).astype(np.float32)
            warped[b] = acc
        warped = warped.reshape(B, C, D, H, W)
        vs += warped
        vq += warped * warped
    var = vq / nv - (vs / nv) ** 2
    return _conv_and_tail(var, dv, reg_w, reg_b, scale=1.0)


def _conv_and_tail(var9, dv, reg_w, reg_b, scale):
    # cost = Conv3d(var, reg_w*scale) + reg_b ; then the softmax tail.
    w = (np.asarray(reg_w, np.float32) * scale)[0]          # [C,3,3,3]
    var9 = np.asarray(var9, np.float32)
    vp = np.pad(var9, ((0, 0), (0, 0), (1, 1), (1, 1), (1, 1)))
    cost = np.zeros((B, D, H, W), np.float32)
    for kd in range(3):
        for ky in range(3):
            for kx in range(3):
                cost += np.einsum(
                    "c,bcdhw->bdhw", w[:, kd, ky, kx],
                    vp[:, :, kd:kd + D, ky:ky + H, kx:kx + W],
                    optimize=True)
    cost = cost + np.float32(np.asarray(reg_b).reshape(-1)[0])
    return _tail(cost.astype(np.float32), dv)


def _tail(cost, dv):
    m = cost.max(axis=1, keepdims=True)
    e = np.exp(cost - m)
    prob = e / e.sum(axis=1, keepdims=True)
    dvf = np.asarray(dv, np.float32)
    depth = (prob * dvf[:, :, None, None]).sum(axis=1)
    pp = np.pad(prob, ((0, 0), (1, 2), (0, 0), (0, 0)))
    psum4 = pp[:, 0:D] + pp[:, 1:D + 1] + pp[:, 2:D + 2] + pp[:, 3:D + 3]
    didx = (prob * np.arange(D, dtype=np.float32)[None, :, None, None]).sum(axis=1)
    didx = np.clip(didx.astype(np.int32), 0, D - 1)
    conf = np.take_along_axis(psum4, didx[:, None], axis=1)[:, 0]
    return depth.astype(np.float32), conf.astype(np.float32)


def _shift_params(proj_matrices, depth_values):
    """Return (ok, s[v-1,b,d,2]) with (sy,sx) uniform shifts, or ok=False."""
    pm = np.asarray(proj_matrices, np.float64)
    dv = np.asarray(depth_values, np.float64)
    refp = pm[:, 0]
    s = np.zeros((V - 1, B, D, 2))
    for v in range(1, V):
        for b in range(B):
            proj = pm[b, v] @ np.linalg.inv(refp[b])
            rot, tr = proj[:3, :3], proj[:3, 3]
            if not np.allclose(rot, np.eye(3), atol=1e-5):
                return False, None
            if abs(tr[2]) > 1e-6 * dv[b].min():
                return False, None
            s[v - 1, b, :, 0] = tr[1] / dv[b]   # sy
            s[v - 1, b, :, 1] = tr[0] / dv[b]   # sx
    # tap windows must stay inside the padded canvas
    if not (s[..., 0].min() >= -(PY - 1) and s[..., 0].max() < PY - 1 and
            s[..., 1].min() >= -PX and s[..., 1].max() < PX - 1):
        return False, None
    return True, s


def _tap_weights(s, grids):
    """wts[vv][b, d, ti] fractional bilinear weight per tap."""
    wts = []
    for vv in range(V - 1):
        wv = np.zeros((B, D, len(grids[vv])), np.float64)
        for b in range(B):
            for d in range(D):
                sy, sx = s[vv, b, d]
                y0, x0 = int(np.floor(sy)), int(np.floor(sx))
                fy, fx = sy - y0, sx - x0
                for ti, (dy, dx) in enumerate(grids[vv]):
                    wy = (1 - fy) if dy == y0 else (fy if dy == y0 + 1 else 0.0)
                    wx = (1 - fx) if dx == x0 else (fx if dx == x0 + 1 else 0.0)
                    wv[b, d, ti] = wy * wx
        wts.append(wv)
    return wts


_CACHE = {}


def _build_program(grids):
    import concourse.mybir as mybir
    from concourse import bacc, tile

    TPG = sum(len(g) for g in grids)
    NTAP = B * NGD * TPG
    nc = bacc.Bacc("TRN2", target_bir_lowering=False, debug=False, num_devices=8)
    F16 = mybir.dt.float16
    F32 = mybir.dt.float32
    fslab = nc.dram_tensor("fslab", [B, V, C, SR * WP], F32, kind="ExternalInput")
    wtab = nc.dram_tensor("wtab", [128, NTAP], F32, kind="ExternalInput")
    wsconv = nc.dram_tensor("wsconv", [128, 54], F32, kind="ExternalInput")
    vout = nc.dram_tensor("vout", [B, D, HS * W], F32, kind="ExternalOutput")
    AL = mybir.AluOpType
    SQ = mybir.ActivationFunctionType.Square

    def v3(ap, xc):
        return ap.rearrange("p (y x) -> p y x", x=xc)

    with tile.TileContext(nc) as tc:
        with tc.tile_pool(name="const", bufs=1) as cpool, \
             tc.tile_pool(name="imgs", bufs=1) as ipool, \
             tc.tile_pool(name="work", bufs=1) as wpool, \
             tc.tile_pool(name="varp", bufs=2) as vpool, \
             tc.tile_pool(name="acc", bufs=2) as apool, \
             tc.tile_pool(name="psum", bufs=4, space="PSUM") as ppool:
            wt = cpool.tile([128, NTAP], F32, tag="wt")
            nc.sync.dma_start(out=wt[:], in_=wtab.ap())
            ws = cpool.tile([128, 54], F32, tag="ws")
            nc.sync.dma_start(out=ws[:], in_=wsconv.ap())
            # stationary [128, 9, 94]: the 9 shift blocks [128, 6] embedded
            # at cols 44..50 of a zero sea; group g uses the free-dim window
            # [44-4g, 94-4g) so matmul emits all 50 depth planes directly.
            wse = cpool.tile([128, 9 * 94], F32, tag="wse")
            nc.any.memset(wse[:], 0.0)
            wsev = v3(wse[:], 94)
            nc.vector.scalar_tensor_tensor(
                out=wsev[:, :, 44:50], in0=v3(ws[:], 6), scalar=1.0,
                in1=wsev[:, :, 44:50], op0=AL.mult, op1=AL.bypass)
            for b in range(B):
                imgs = []
                for v in range(V):
                    it = ipool.tile([128, SR * WP], F32, tag=f"i{v}")
                    for j in range(4):
                        nc.sync.dma_start(out=it[32 * j:32 * j + 32, :],
                                          in_=fslab.ap()[b, v])
                    imgs.append(it)
                costacc = apool.tile([D + 2, HS * W], F32, tag="costacc")
                cview = v3(costacc[:], W)
                nc.any.memset(costacc[:], 0.0)
                for g in range(NGD):
                    w1 = wpool.tile([128, VR * W], F32, tag="w1")
                    w2 = wpool.tile([128, VR * W], F32, tag="w2")
                    av = wpool.tile([128, VR * W], F32, tag="av")
                    vt = wpool.tile([128, VR * W], F32, tag="vt")
                    col0 = (b * NGD + g) * TPG
                    for vv, (dst, srci) in enumerate(((w1, imgs[1]), (w2, imgs[2]))):
                        off = col0 + (0 if vv == 0 else len(grids[0]))
                        for ti, (dy, dx) in enumerate(grids[vv]):
                            sap = v3(srci[:], WP)[:, dy + PY - 1:dy + PY - 1 + VR,
                                                  dx + PX:dx + PX + W]
                            nc.vector.scalar_tensor_tensor(
                                out=v3(dst[:], W), in0=sap,
                                scalar=wt[:, off + ti:off + ti + 1],
                                in1=v3(dst[:], W),
                                op0=AL.mult,
                                op1=(AL.bypass if ti == 0 else AL.add))
                    i0 = v3(imgs[0][:], WP)[:, PY - 1:PY - 1 + VR, PX:PX + W]
                    # av = I0 - W1 ; vt = I0 - W2 ; w1 <- W1 - W2
                    nc.vector.scalar_tensor_tensor(
                        out=v3(av[:], W), in0=v3(w1[:], W), scalar=-1.0, in1=i0,
                        op0=AL.mult, op1=AL.add)
                    nc.vector.scalar_tensor_tensor(
                        out=v3(vt[:], W), in0=v3(w2[:], W), scalar=-1.0, in1=i0,
                        op0=AL.mult, op1=AL.add)
                    nc.vector.scalar_tensor_tensor(
                        out=w1[:], in0=w2[:], scalar=-1.0, in1=w1[:],
                        op0=AL.mult, op1=AL.add)
                    nc.scalar.activation(out=av[:], in_=av[:], func=SQ)
                    nc.scalar.activation(out=vt[:], in_=vt[:], func=SQ)
                    nc.scalar.activation(out=w1[:], in_=w1[:], func=SQ)
                    nc.vector.scalar_tensor_tensor(
                        out=av[:], in0=vt[:], scalar=1.0, in1=av[:],
                        op0=AL.mult, op1=AL.add)
                    var = vpool.tile([128, VR * VCW], F32, tag="var")
                    nc.any.memset(var[:], 0.0)
                    nc.vector.scalar_tensor_tensor(
                        out=v3(var[:], VCW)[:, :, 1:1 + W], in0=w1[:].rearrange(
                            "p (y x) -> p y x", x=W),
                        scalar=1.0, in1=v3(av[:], W),
                        op0=AL.mult, op1=AL.add)
                    varv = v3(var[:], VCW)
                    for ch in range(NCH):
                        ps = ppool.tile([D + 2, HS * CHW], F32, tag="ps")
                        for si, (ky, kx) in enumerate(
                                (ky, kx) for ky in range(3) for kx in range(3)):
                            rhs = varv[:, ky:ky + HS,
                                       kx + CHW * ch:kx + CHW * ch + CHW]
                            nc.tensor.matmul(
                                ps[:],
                                wsev[:, si, 44 - 4 * g:94 - 4 * g],
                                rhs, start=(si == 0), stop=(si == 8))
                        nc.vector.scalar_tensor_tensor(
                            out=cview[:, :, CHW * ch:CHW * ch + CHW],
                            in0=v3(ps[:], CHW), scalar=1.0,
                            in1=cview[:, :, CHW * ch:CHW * ch + CHW],
                            op0=AL.mult, op1=AL.add)
                nc.sync.dma_start(out=vout.ap()[b], in_=costacc[1:D + 1, :])
    nc.finalize()
    return nc, TPG, NTAP


def _edge_cost(features, s, grids, reg_w):
    """Exact cost rows (no reg_b) at h=0 and h=127: two [B,D,W] arrays."""
    f = np.asarray(features, np.float32)
    w9 = (np.asarray(reg_w, np.float32) / 9.0)[0]      # [C,3,3,3]
    wts = _tap_weights(s, grids)
    rows = [0, 1, H - 2, H - 1]
    var4 = np.zeros((B, C, D, 4, W), np.float32)
    for b in range(B):
        i0 = f[0, b][:, rows, :]                       # [C,4,W]
        wv = np.zeros((2, C, D, 4, W), np.float32)
        for vv in range(V - 1):
            img = np.zeros((C, H + 2 * PY, W + 2 * PX), np.float32)
            img[:, PY:PY + H, PX:PX + W] = f[vv + 1, b]
            for ti, (dy, dx) in enumerate(grids[vv]):
                sl = img[:, [r + dy + PY for r in rows], dx + PX:dx + PX + W]
                wv[vv] += wts[vv][b][None, :, ti, None, None] * sl[:, None]
        d0 = i0[:, None] - wv[0]
        d1 = i0[:, None] - wv[1]
        d2 = wv[0] - wv[1]
        var4[b] = d0 * d0 + d1 * d1 + d2 * d2
    vp = np.zeros((B, C, D + 2, 4, W + 2), np.float32)
    vp[:, :, 1:D + 1, :, 1:W + 1] = var4
    c0 = np.zeros((B, D, W), np.float32)
    c1 = np.zeros((B, D, W), np.float32)
    for kd in range(3):
        for kx in range(3):
            for ky in (1, 2):      # h=0: var row (ky-1); ky=0 reads zero pad
                c0 += np.einsum("c,bcdw->bdw", w9[:, kd, ky, kx],
                                vp[:, :, kd:kd + D, ky - 1, kx:kx + W])
            for ky in (0, 1):      # h=127: var row (126+ky); ky=2 reads pad
                c1 += np.einsum("c,bcdw->bdw", w9[:, kd, ky, kx],
                                vp[:, :, kd:kd + D, 2 + ky, kx:kx + W])
    return c0, c1


def kernel(features, proj_matrices, depth_values, num_depth, reg_w, reg_b):
    features = np.asarray(features, np.float32)
    dv = np.asarray(depth_values, np.float32)
    ok, s = _shift_params(proj_matrices, depth_values)
    if not ok or int(num_depth) != D:
        return _host_reference(features, proj_matrices, depth_values,
                               num_depth, reg_w, reg_b)

    # tap grids: union of (dy,dx) integer offsets per view over all (b,d)
    grids = []
    for vv in range(V - 1):
        taps = set()
        for b in range(B):
            for d in range(D):
                sy, sx = s[vv, b, d]
                y0, x0 = int(np.floor(sy)), int(np.floor(sx))
                for a in (0, 1):
                    for c2 in (0, 1):
                        taps.add((y0 + a, x0 + c2))
        grids.append(sorted(taps))

    key = tuple(tuple(g) for g in grids)
    if key not in _CACHE:
        _CACHE[key] = _build_program(grids)
    nc, TPG, NTAP = _CACHE[key]

    from concourse import bass_utils

    # per-core 24-row fp16 slabs of the zero-padded canvas
    can = np.zeros((B, V, C, H + 2 * PY, WP), np.float32)
    can[:, :, :, PY:PY + H, PX:PX + W] = features.transpose(1, 0, 2, 3, 4)

    # tap weight table [128, NTAP] fp16 (identical on all cores):
    # partition (j,c) row j*32+c -> depth 4g+j; column (b*NGD+g)*TPG + tap
    wts = _tap_weights(s, grids)
    wtabv = np.zeros((128, NTAP), np.float32)
    for b in range(B):
        for g in range(NGD):
            col0 = (b * NGD + g) * TPG
            off = 0
            for vv in range(V - 1):
                for ti in range(len(grids[vv])):
                    for j in range(4):
                        wtabv[32 * j:32 * j + 32, col0 + off + ti] = \
                            wts[vv][b, 4 * g + j, ti]
                off += len(grids[vv])

    # stationary conv matrix [128, 9*6] fp16: col s*6+e couples var depth
    # j (partition block) to output plane 4g+e-1 via kernel tap kd=j-e+2
    w9 = (np.asarray(reg_w, np.float32) / 9.0)[0]          # [C,3,3,3]
    wsv = np.zeros((128, 54), np.float32)
    for si, (ky, kx) in enumerate((ky, kx) for ky in range(3) for kx in range(3)):
        for j in range(4):
            for e in range(6):
                kd = j - e + 2
                if 0 <= kd <= 2:
                    wsv[32 * j:32 * j + 32, si * 6 + e] = w9[:, kd, ky, kx]

    in_maps = []
    for k in range(8):
        slab = np.ascontiguousarray(
            can[:, :, :, HS * k:HS * k + SR, :]).reshape(B, V, C, SR * WP)
        in_maps.append({"fslab": slab, "wtab": wtabv, "wsconv": wsv})

    import time as _time
    res = bass_utils.run_bass_kernel_spmd(nc, in_maps, list(range(8)))
    t0 = _time.time()
    res = bass_utils.run_bass_kernel_spmd(nc, in_maps, list(range(8)))
    dev_wall_ns = int((_time.time() - t0) * 1e9)
    global LAST_EXEC_NS
    LAST_EXEC_NS = res.exec_time_ns or dev_wall_ns
    outs = res.results

    rb = np.float32(np.asarray(reg_b).reshape(-1)[0])
    cost = np.empty((B, D, H, W), np.float32)
    for k in range(8):
        cost[:, :, HS * k:HS * k + HS, :] = np.asarray(
            outs[k]["vout"], np.float32).reshape(B, D, HS, W)
    cost += rb
    c0, c1 = _edge_cost(features, s, grids, reg_w)
    cost[:, :, 0, :] = c0 + rb
    cost[:, :, H - 1, :] = c1 + rb
    return _tail(cost, dv)


LAST_EXEC_NS = 0
